# revision 23
# baseline (speedup 1.0000x reference)
"""DCRNN (diffusion-conv GRU encoder-decoder) Trainium2 kernel.

Strategy (8 NeuronCores, data-parallel over batch B=32 -> 4/core):
  * Everything SBUF-resident: support (as S^T and (S^2)^T, fp16), all
    weights, hidden states. HBM traffic is only the tiny per-step input
    rows and output rows.
  * Commuted gconv: for Chebyshev terms x0,x1,x2 and per-hop weights
    W0,W1,W2 (torch row-interleaved f*3+k),
        gconv(v) = v(W0-W2) + S (v W1) + S^2 (v 2W2) + b
    so the feature projection (tiny K<=128 matmul) happens FIRST and the
    two diffusion applications become ONE fused 16-K-chunk PE sweep over
    [S^T | S2^T] with no mid-sweep dependency. S^2 is precomputed on host.
  * Layouts: node-major (node-chunk on partitions) for diffusion sweeps
    and all elementwise ops; feature-major x0^T tiles (built with xbar
    DMA-transposes, off the PE) serve as stationary lhsT for the
    projections. PE matmuls are fp16 (1 col/cycle) with fp32 PSUM.
"""

import numpy as np

SEQ, HOR, N, BTOT, U, K = 12, 12, 1024, 32, 64, 2
NCORES = 8
B = BTOT // NCORES          # 4 batch / core
NCH = N // 128              # 8 node chunks

_CACHE = {}


def _split_w(W, F):
    W = np.asarray(W, np.float32)
    return W[0::3], W[1::3], W[2::3]


def _pack_gate(Wg, F, dx, out_o, bias=None):
    """(3F, O) torch-interleaved -> (128, 3*O) fp16 [Wq | W1 | 2W2] with
    x-features at rows 0:dx, h-features at rows 64:128, and (layer-0
    cells only) the bias at row 32, matched by a constant-1.0 feature
    row in the lhsT tiles."""
    W0, W1, W2 = _split_w(Wg, F)
    cat = np.concatenate([W0 - W2, W1, 2.0 * W2], axis=1)  # (F, 3*O)
    out = np.zeros((128, 3 * out_o), np.float32)
    out[0:dx] = cat[0:dx]
    out[64:128] = cat[dx:dx + 64]
    if bias is not None:
        out[32, 0:out_o] = np.asarray(bias, np.float32)
    return out.astype(np.float16)


def _prep_host(inputs, support, params):
    S = np.asarray(support, np.float32)
    S2 = S @ S
    ST = np.ascontiguousarray(S.T).reshape(NCH, 128, NCH, 128)
    ST = np.ascontiguousarray(ST.transpose(1, 0, 2, 3)).reshape(128, NCH * NCH * 128)
    S2T = np.ascontiguousarray(S2.T).reshape(NCH, 128, NCH, 128)
    S2T = np.ascontiguousarray(S2T.transpose(1, 0, 2, 3)).reshape(128, NCH * NCH * 128)

    cells = [params['enc'][0], params['enc'][1], params['dec'][0], params['dec'][1]]
    dxs = [2, 64, 1, 64]
    WG = np.stack([_pack_gate(c[0], dx + 64, dx, 128,
                              bias=(c[1] if dx <= 2 else None))
                   for c, dx in zip(cells, dxs)], 1)
    WC = np.stack([_pack_gate(c[2], dx + 64, dx, 64,
                              bias=(c[3] if dx <= 2 else None))
                   for c, dx in zip(cells, dxs)], 1)
    BG = np.stack([np.broadcast_to(np.asarray(c[1], np.float32), (128, 128))
                   for c in cells], 1)
    BC = np.stack([np.broadcast_to(np.asarray(c[3], np.float32), (128, 64))
                   for c in cells], 1)
    PW = np.zeros((128, 1), np.float32)
    PW[64:128, 0] = np.asarray(params['proj_W'], np.float32)[:, 0]
    PB = np.asarray(params['proj_b'], np.float32).reshape(1, 1)

    # encoder inputs, feature-major per (t, b): (12, B, 2, N) per core
    x = np.asarray(inputs, np.float32).reshape(SEQ, BTOT, N, 2)
    xT = np.ascontiguousarray(x.transpose(0, 1, 3, 2)).astype(np.float16)

    shared = {
        'ST': ST.astype(np.float16), 'S2T': S2T.astype(np.float16),
        'WG': np.ascontiguousarray(WG).reshape(128, -1),
        'WC': np.ascontiguousarray(WC).reshape(128, -1),
        'BG': np.ascontiguousarray(BG).reshape(128, -1),
        'BC': np.ascontiguousarray(BC).reshape(128, -1),
        'PW': PW.astype(np.float16), 'PB': PB,
    }
    in_maps = []
    for c in range(NCORES):
        m = dict(shared)
        m['xT'] = np.ascontiguousarray(xT[:, c * B:(c + 1) * B])
        in_maps.append(m)
    return in_maps


def _build_program_dbg(n_steps):
    return _build_program(n_steps, dbg=True)


def _build_program(n_steps=SEQ + HOR, dbg=False):
    import concourse.bass as bass
    import concourse.tile as tile
    from concourse import mybir

    f32, f16 = mybir.dt.float32, mybir.dt.float16
    AF = mybir.ActivationFunctionType

    nc = bass.Bass()
    dST = nc.declare_dram_parameter("ST", [128, NCH * NCH * 128], f16, isOutput=False)
    dS2T = nc.declare_dram_parameter("S2T", [128, NCH * NCH * 128], f16, isOutput=False)
    dWG = nc.declare_dram_parameter("WG", [128, 4 * 384], f16, isOutput=False)
    dWC = nc.declare_dram_parameter("WC", [128, 4 * 192], f16, isOutput=False)
    dBG = nc.declare_dram_parameter("BG", [128, 4 * 128], f32, isOutput=False)
    dBC = nc.declare_dram_parameter("BC", [128, 4 * 64], f32, isOutput=False)
    dPW = nc.declare_dram_parameter("PW", [128, 1], f16, isOutput=False)
    dPB = nc.declare_dram_parameter("PB", [1, 1], f32, isOutput=False)
    dxT = nc.declare_dram_parameter("xT", [SEQ, B, 2, N], f16, isOutput=False)
    dY = nc.declare_dram_parameter("Y", [HOR, B, N], f32, isOutput=True)
    if dbg:
        dDH = nc.declare_dram_parameter("DH", [128, B * NCH * 2 * U], f16, isOutput=True)
        dDRU = nc.declare_dram_parameter("DRU", [128, NCH * B * 128], f16, isOutput=True)
        dDPG = nc.declare_dram_parameter("DPG", [128, NCH * B * 3 * 128], f16, isOutput=True)
        dDX1 = nc.declare_dram_parameter("DX1", [128, N], f16, isOutput=True)

    def bc_b(ap, nb=B):
        """insert a stride-0 batch dim after the partition dim"""
        return bass.AP(tensor=ap.tensor, offset=ap.offset,
                       ap=[list(ap.ap[0]), [0, nb]] + [list(a) for a in ap.ap[1:]])

    def bc_b2(ap, nb=B):
        """stride-0 (chunk-pair, batch) dims after the partition dim"""
        return bass.AP(tensor=ap.tensor, offset=ap.offset,
                       ap=[list(ap.ap[0]), [0, 2], [0, nb]] + [list(a) for a in ap.ap[1:]])

    with tile.TileContext(nc) as tc:
        import contextlib
        with contextlib.ExitStack() as ctx:
            const = ctx.enter_context(tc.tile_pool(name="const", bufs=1))
            state = ctx.enter_context(tc.tile_pool(name="state", bufs=1))
            pkps = ctx.enter_context(tc.tile_pool(name="pkps", bufs=4, space="PSUM"))
            swps = ctx.enter_context(tc.tile_pool(name="swps", bufs=2, space="PSUM"))
            pjps = ctx.enter_context(tc.tile_pool(name="pjps", bufs=2, space="PSUM"))
            tmp = ctx.enter_context(tc.tile_pool(name="tmp", bufs=3))

            sST = const.tile([128, NCH, NCH, 128], f16)
            nc.sync.dma_start(out=sST, in_=dST[:])
            sS2T = const.tile([128, NCH, NCH, 128], f16)
            nc.sync.dma_start(out=sS2T, in_=dS2T[:])
            sWG = const.tile([128, 4, 384], f16)
            nc.sync.dma_start(out=sWG, in_=dWG[:])
            sWC = const.tile([128, 4, 192], f16)
            nc.sync.dma_start(out=sWC, in_=dWC[:])
            sBG = const.tile([128, 4, 128], f32)
            nc.sync.dma_start(out=sBG, in_=dBG[:])
            sBC = const.tile([128, 4, 64], f32)
            nc.sync.dma_start(out=sBC, in_=dBC[:])
            sPW = const.tile([128, 1], f16)
            nc.sync.dma_start(out=sPW, in_=dPW[:])
            sPB = const.tile([1, 1], f32)
            nc.sync.dma_start(out=sPB, in_=dPB[:])

            h_all = state.tile([128, B, NCH, 2, U], f16, tag="h_all")
            ru = state.tile([128, NCH, B, 128], f16, tag="ru")
            cc = state.tile([128, NCH, B, U], f16, tag="cc")
            rh = state.tile([128, 2, NCH, 2, U], f16, tag="rh")
            rhx = state.tile([128, B, NCH, 2, U], f16, tag="rhx")
            p_g = state.tile([128, NCH, B, 3, 128], f16, tag="p_g")
            p_c = state.tile([128, NCH, B, 3, 64], f16, tag="p_c")
            x0T_l0 = [state.tile([128, NCH, 128], f16, tag=f"x0T_l0_{b}", name=f"x0T_l0_{b}") for b in range(B)]
            x0T_l1 = [state.tile([128, NCH, 128], f16, tag=f"x0T_l1_{b}", name=f"x0T_l1_{b}") for b in range(B)]
            x0cT_l0 = [state.tile([128, NCH, 128], f16, tag=f"x0cT_l0_{b}", name=f"x0cT_l0_{b}") for b in range(B)]
            x0cT_l1 = [state.tile([128, NCH, 128], f16, tag=f"x0cT_l1_{b}", name=f"x0cT_l1_{b}") for b in range(B)]
            rhT_pair = [state.tile([128, NCH, 128], f16, tag=f"rhT_{p}", name=f"rhT_{p}") for p in range(2)]
            yT = [state.tile([1, N], f16, tag=f"yT_{b}", name=f"yT_{b}") for b in range(B)]
            ysb = [state.tile([1, N], f32, tag=f"ysb_{b}", name=f"ysb_{b}") for b in range(B)]

            for t_ in ([h_all] + x0T_l0 + x0T_l1 + x0cT_l0 + x0cT_l1 + yT):
                nc.vector.memset(t_, 0.0)
            for t_ in x0T_l0 + x0cT_l0:
                nc.vector.memset(t_[32:33, :, :], 1.0)

            def gconv(x0T, psb, wS, bS, wi, O, act, outt, skip_bias=False):
                """pk projections + fused diffusion sweep + combine.
                x0T: per-b feature-major lhsT tiles; psb: SBUF (128,NCH,B,3,O)
                fp16 landing pad; wS/bS: weight/bias const tiles; O: 128|64;
                act: Sigmoid|Tanh; outt: (128,NCH,B,O) fp16 output."""
                for half in range(2):
                    for b in range(B):
                        for mc in range(half * 4, half * 4 + 4):
                            ps = pkps.tile([128, 3 * O], f32, tag="pk")
                            nc.tensor.matmul(ps, x0T[b][:, mc, :],
                                             wS[:, wi, :], start=True, stop=True)
                            if (b * 4 + mc) % 2 == 0:
                                nc.scalar.copy(psb[:, mc, b, :, :], ps)
                            else:
                                nc.vector.tensor_copy(psb[:, mc, b, :, :], ps)
                for mc in range(NCH):
                    sw = swps.tile([128, B, O], f32, tag="sw")
                    for kk in range(2 * NCH):
                        lhs = (sST if kk < NCH else sS2T)[:, kk % NCH, mc, :]
                        rhs = psb[:, kk % NCH, :, 1 if kk < NCH else 2, :]
                        nc.tensor.matmul(sw, lhs, rhs, start=(kk == 0),
                                         stop=(kk == 2 * NCH - 1))
                    t1 = tmp.tile([128, B, O], f32, tag="t1")
                    nc.vector.tensor_add(t1, sw, psb[:, mc, :, 0, :])
                    if skip_bias:
                        nc.scalar.activation(outt[:, mc, :, :], t1, act)
                    else:
                        t2 = tmp.tile([128, B, O], f32, tag="t2")
                        nc.vector.tensor_add(t2, t1, bc_b(bS[:, wi, :]))
                        nc.scalar.activation(outt[:, mc, :, :], t2, act)

            def cell(l, step, tail):
                enc = step < SEQ
                wi = (0 if enc else 2) + l
                dx = (2 if enc else 1) if l == 0 else 64
                x0T = x0T_l0 if l == 0 else x0T_l1
                x0cT = x0cT_l0 if l == 0 else x0cT_l1
                # ---- gate ----
                gconv(x0T, p_g, sWG, sBG, wi, 128, AF.Sigmoid, ru, skip_bias=(l == 0))
                # ---- candidate input r*h (+ transposes), per chunk-half so
                # the first half is ready while the gate sweep still runs ----
                if l == 0:
                    for half in range(2):
                        hs = slice(half * 4, half * 4 + 4)
                        for pr in range(2):
                            for b in (2 * pr, 2 * pr + 1):
                                nc.vector.tensor_mul(
                                    rh[:, b // 2, hs, b % 2, :],
                                    ru[:, hs, b, 0:U], h_all[:, b, hs, 0, :])
                            nc.sync.dma_start_transpose(
                                rhT_pair[pr][:, hs, :], rh[:, pr, hs, :, :])
                            for b in (2 * pr, 2 * pr + 1):
                                nc.gpsimd.tensor_copy(
                                    x0cT_l0[b][64:128, hs, :],
                                    rhT_pair[pr][(b % 2) * 64:(b % 2) * 64 + 64, hs, :])
                    for b in range(B):
                        nc.gpsimd.tensor_copy(x0cT_l0[b][0:dx, :, :],
                                              x0T_l0[b][0:dx, :, :])
                else:
                    for half in range(2):
                        hs = slice(half * 4, half * 4 + 4)
                        for b in range(B):
                            nc.gpsimd.tensor_copy(rhx[:, b, hs, 0, :],
                                                  h_all[:, b, hs, 0, :])
                            nc.vector.tensor_mul(rhx[:, b, hs, 1, :],
                                                 ru[:, hs, b, 0:U],
                                                 h_all[:, b, hs, 1, :])
                            nc.sync.dma_start_transpose(
                                x0cT_l1[b][:, hs, :], rhx[:, b, hs, :, :])
                # ---- candidate ----
                gconv(x0cT, p_c, sWC, sBC, wi, 64, AF.Tanh, cc, skip_bias=(l == 0))
                # ---- h' = c + u*(h-c), per (b, chunk-half): the first half
                # finalizes+transposes while the cand sweep's tail runs ----
                for half in range(2):
                    hs = slice(half * 4, half * 4 + 4)
                    for b in range(B):
                        hsl = h_all[:, b, hs, l, :]
                        usl = ru[:, hs, b, U:128]
                        csl = cc[:, hs, b, :]
                        d = tmp.tile([128, 4, U], f32, tag="fd")
                        nc.vector.tensor_sub(d, hsl, csl)
                        e = tmp.tile([128, 4, U], f32, tag="fe")
                        nc.vector.tensor_mul(e, d, usl)
                        nc.vector.tensor_add(hsl, e, csl)
                        tail(b, half, hs)

            for step in range(n_steps):
                enc = step < SEQ
                if enc:
                    for b in range(B):
                        nc.sync.dma_start(out=x0T_l0[b][0:2, :, :], in_=dxT[step, b].rearrange('f (c n) -> f c n', c=NCH))
                else:
                    for b in range(B):
                        nc.vector.tensor_copy(x0T_l0[b][0:1, :, :], yT[b].rearrange('o (c n) -> o c n', c=NCH))

                def tail0(b, half, hs):
                    # x0T_l1 <- [h'_l0(s)^T ; h_l1(s-1)^T]: l1 gate input
                    nc.sync.dma_start_transpose(x0T_l1[b][:, hs, :],
                                                h_all[:, b, hs, :, :])

                cell(0, step, tail0)

                td = step - SEQ

                def tail1(b, half, hs):
                    if not enc:
                        # refresh to get h'_l1(s)^T for the projection
                        nc.sync.dma_start_transpose(x0T_l1[b][:, hs, :],
                                                    h_all[:, b, hs, :, :])
                        pp = pjps.tile([1, 512], f32, tag="pj")
                        nc.tensor.matmul(pp, sPW, x0T_l1[b][:, hs, :],
                                         start=True, stop=True)
                        nc.vector.tensor_scalar_add(
                            ysb[b][:, half * 512:(half + 1) * 512], pp, sPB)
                        nc.gpsimd.tensor_copy(yT[b][:, half * 512:(half + 1) * 512],
                                              ysb[b][:, half * 512:(half + 1) * 512])
                        if half == 1:
                            nc.sync.dma_start(out=dY[td, b:b + 1, :], in_=ysb[b])
                    # h'_l0(s)^T for next step's l0 gate
                    nc.gpsimd.tensor_copy(x0T_l0[b][64:128, hs, :],
                                           x0T_l1[b][0:64, hs, :])

                cell(1, step, tail1)

            if dbg:
                nc.sync.dma_start(out=dDH[:], in_=h_all)
                nc.sync.dma_start(out=dDRU[:], in_=ru)
                nc.sync.dma_start(out=dDPG[:], in_=p_g)
                nc.sync.dma_start(out=dDX1[:], in_=x0T_l1[0])

    _fix_multi_waits(nc)
    return nc


def _fix_multi_waits(nc):
    """This container's walrus accepts only ONE sync-wait per CTRL/DMA
    instruction encoding; hoist extra waits onto same-engine NoOps."""
    from concourse import mybir
    for bb in nc.main_func.blocks:
        new = []
        dirty = False
        for inst in bb.instructions:
            si = inst.sync_info
            if si is not None and len(si.on_wait) > 1:
                waits = list(si.on_wait)
                for j, w in enumerate(waits[:-1]):
                    nop = mybir.InstNoOp(name=f"{inst.name}-wsplit{j}", ins=[], outs=[])
                    nop.engine = inst.engine
                    nop.sync_info = mybir.SyncInfo(on_wait=[w], on_update=[])
                    nc.register_instruction(nop, overwrite=True)
                    new.append(nop)
                si.on_wait = waits[-1:]
                dirty = True
            new.append(inst)
        if dirty:
            bb.instructions = new


def kernel(inputs, support, params, trace=False):
    from concourse.bass_utils import run_bass_kernel_spmd

    key = 'prog'
    if key not in _CACHE:
        _CACHE[key] = _build_program()
    nc = _CACHE[key]

    in_maps = _prep_host(inputs, support, params)
    res = run_bass_kernel_spmd(nc, in_maps, list(range(NCORES)), trace=trace)
    if trace:
        kernel.last_result = res
    out = np.concatenate([np.asarray(res.results[c]["Y"]) for c in range(NCORES)],
                         axis=1)
    return out.reshape(HOR, BTOT, N).astype(np.float32)


# revision 24
# speedup vs baseline: 1.0928x; 1.0928x over previous
"""DCRNN (diffusion-conv GRU encoder-decoder) Trainium2 kernel.

Strategy (8 NeuronCores, data-parallel over batch B=32 -> 4/core):
  * Everything SBUF-resident: support (as S^T and (S^2)^T, fp16), all
    weights, hidden states. HBM traffic is only the tiny per-step input
    rows and output rows.
  * Commuted gconv: for Chebyshev terms x0,x1,x2 and per-hop weights
    W0,W1,W2 (torch row-interleaved f*3+k),
        gconv(v) = v(W0-W2) + S (v W1) + S^2 (v 2W2) + b
    so the feature projection (tiny K<=128 matmul) happens FIRST and the
    two diffusion applications become ONE fused 16-K-chunk PE sweep over
    [S^T | S2^T] with no mid-sweep dependency. S^2 is precomputed on host.
  * Layouts: node-major (node-chunk on partitions) for diffusion sweeps
    and all elementwise ops; feature-major x0^T tiles (built with xbar
    DMA-transposes, off the PE) serve as stationary lhsT for the
    projections. PE matmuls are fp16 (1 col/cycle) with fp32 PSUM.
"""

import numpy as np

SEQ, HOR, N, BTOT, U, K = 12, 12, 1024, 32, 64, 2
NCORES = 8
B = BTOT // NCORES          # 4 batch / core
NCH = N // 128              # 8 node chunks

_CACHE = {}


def _split_w(W, F):
    W = np.asarray(W, np.float32)
    return W[0::3], W[1::3], W[2::3]


def _pack_gate(Wg, F, dx, out_o, bias=None):
    """(3F, O) torch-interleaved -> (128, 3*O) fp16 [Wq | W1 | 2W2] with
    x-features at rows 0:dx, h-features at rows 64:128, and (layer-0
    cells only) the bias at row 32, matched by a constant-1.0 feature
    row in the lhsT tiles."""
    W0, W1, W2 = _split_w(Wg, F)
    cat = np.concatenate([W0 - W2, W1, 2.0 * W2], axis=1)  # (F, 3*O)
    out = np.zeros((128, 3 * out_o), np.float32)
    out[0:dx] = cat[0:dx]
    out[64:128] = cat[dx:dx + 64]
    if bias is not None:
        out[32, 0:out_o] = np.asarray(bias, np.float32)
    return out.astype(np.float16)


def _prep_host(inputs, support, params):
    S = np.asarray(support, np.float32)
    S2 = S @ S
    ST = np.ascontiguousarray(S.T).reshape(NCH, 128, NCH, 128)
    ST = np.ascontiguousarray(ST.transpose(1, 0, 2, 3)).reshape(128, NCH * NCH * 128)
    S2T = np.ascontiguousarray(S2.T).reshape(NCH, 128, NCH, 128)
    S2T = np.ascontiguousarray(S2T.transpose(1, 0, 2, 3)).reshape(128, NCH * NCH * 128)

    cells = [params['enc'][0], params['enc'][1], params['dec'][0], params['dec'][1]]
    dxs = [2, 64, 1, 64]
    WG = np.stack([_pack_gate(c[0], dx + 64, dx, 128,
                              bias=(c[1] if dx <= 2 else None))
                   for c, dx in zip(cells, dxs)], 1)
    WC = np.stack([_pack_gate(c[2], dx + 64, dx, 64,
                              bias=(c[3] if dx <= 2 else None))
                   for c, dx in zip(cells, dxs)], 1)
    BG = np.stack([np.broadcast_to(np.asarray(c[1], np.float32), (128, 128))
                   for c in cells], 1)
    BC = np.stack([np.broadcast_to(np.asarray(c[3], np.float32), (128, 64))
                   for c in cells], 1)
    PW = np.zeros((128, 1), np.float32)
    PW[64:128, 0] = np.asarray(params['proj_W'], np.float32)[:, 0]
    PB = np.asarray(params['proj_b'], np.float32).reshape(1, 1)

    # encoder inputs, feature-major per (t, b): (12, B, 2, N) per core
    x = np.asarray(inputs, np.float32).reshape(SEQ, BTOT, N, 2)
    xT = np.ascontiguousarray(x.transpose(0, 1, 3, 2)).astype(np.float16)

    shared = {
        'ST': ST.astype(np.float16), 'S2T': S2T.astype(np.float16),
        'WG': np.ascontiguousarray(WG).reshape(128, -1),
        'WC': np.ascontiguousarray(WC).reshape(128, -1),
        'BG': np.ascontiguousarray(BG).reshape(128, -1),
        'BC': np.ascontiguousarray(BC).reshape(128, -1),
        'PW': PW.astype(np.float16), 'PB': PB,
    }
    in_maps = []
    for c in range(NCORES):
        m = dict(shared)
        m['xT'] = np.ascontiguousarray(xT[:, c * B:(c + 1) * B])
        in_maps.append(m)
    return in_maps


def _build_program_dbg(n_steps):
    return _build_program(n_steps, dbg=True)


def _build_program(n_steps=SEQ + HOR, dbg=False):
    import concourse.bass as bass
    import concourse.tile as tile
    from concourse import mybir

    f32, f16 = mybir.dt.float32, mybir.dt.float16
    AF = mybir.ActivationFunctionType

    nc = bass.Bass()
    dST = nc.declare_dram_parameter("ST", [128, NCH * NCH * 128], f16, isOutput=False)
    dS2T = nc.declare_dram_parameter("S2T", [128, NCH * NCH * 128], f16, isOutput=False)
    dWG = nc.declare_dram_parameter("WG", [128, 4 * 384], f16, isOutput=False)
    dWC = nc.declare_dram_parameter("WC", [128, 4 * 192], f16, isOutput=False)
    dBG = nc.declare_dram_parameter("BG", [128, 4 * 128], f32, isOutput=False)
    dBC = nc.declare_dram_parameter("BC", [128, 4 * 64], f32, isOutput=False)
    dPW = nc.declare_dram_parameter("PW", [128, 1], f16, isOutput=False)
    dPB = nc.declare_dram_parameter("PB", [1, 1], f32, isOutput=False)
    dxT = nc.declare_dram_parameter("xT", [SEQ, B, 2, N], f16, isOutput=False)
    dY = nc.declare_dram_parameter("Y", [HOR, B, N], f32, isOutput=True)
    if dbg:
        dDH = nc.declare_dram_parameter("DH", [128, B * NCH * 2 * U], f16, isOutput=True)
        dDRU = nc.declare_dram_parameter("DRU", [128, NCH * B * 128], f16, isOutput=True)
        dDPG = nc.declare_dram_parameter("DPG", [128, NCH * B * 3 * 128], f16, isOutput=True)
        dDX1 = nc.declare_dram_parameter("DX1", [128, N], f16, isOutput=True)

    def bc_b(ap, nb=B):
        """insert a stride-0 batch dim after the partition dim"""
        return bass.AP(tensor=ap.tensor, offset=ap.offset,
                       ap=[list(ap.ap[0]), [0, nb]] + [list(a) for a in ap.ap[1:]])

    def bc_b2(ap, nb=B):
        """stride-0 (chunk-pair, batch) dims after the partition dim"""
        return bass.AP(tensor=ap.tensor, offset=ap.offset,
                       ap=[list(ap.ap[0]), [0, 2], [0, nb]] + [list(a) for a in ap.ap[1:]])

    with tile.TileContext(nc) as tc:
        import contextlib
        with contextlib.ExitStack() as ctx:
            const = ctx.enter_context(tc.tile_pool(name="const", bufs=1))
            state = ctx.enter_context(tc.tile_pool(name="state", bufs=1))
            pkps = ctx.enter_context(tc.tile_pool(name="pkps", bufs=4, space="PSUM"))
            swps = ctx.enter_context(tc.tile_pool(name="swps", bufs=2, space="PSUM"))
            pjps = ctx.enter_context(tc.tile_pool(name="pjps", bufs=2, space="PSUM"))
            tmp = ctx.enter_context(tc.tile_pool(name="tmp", bufs=3))

            sST = const.tile([128, NCH, NCH, 128], f16)
            nc.sync.dma_start(out=sST, in_=dST[:])
            sS2T = const.tile([128, NCH, NCH, 128], f16)
            nc.sync.dma_start(out=sS2T, in_=dS2T[:])
            sWG = const.tile([128, 4, 384], f16)
            nc.sync.dma_start(out=sWG, in_=dWG[:])
            sWC = const.tile([128, 4, 192], f16)
            nc.sync.dma_start(out=sWC, in_=dWC[:])
            sBG = const.tile([128, 4, 128], f32)
            nc.sync.dma_start(out=sBG, in_=dBG[:])
            sBC = const.tile([128, 4, 64], f32)
            nc.sync.dma_start(out=sBC, in_=dBC[:])
            sPW = const.tile([128, 1], f16)
            nc.sync.dma_start(out=sPW, in_=dPW[:])
            sPB = const.tile([1, 1], f32)
            nc.sync.dma_start(out=sPB, in_=dPB[:])

            h_all = state.tile([128, B, NCH, 2, U], f16, tag="h_all")
            ru = state.tile([128, NCH, B, 128], f16, tag="ru")
            cc = state.tile([128, NCH, B, U], f16, tag="cc")
            rh = state.tile([128, 2, NCH, 2, U], f16, tag="rh")
            rhx = state.tile([128, B, NCH, 2, U], f16, tag="rhx")
            p_g = state.tile([128, NCH, B, 3, 128], f16, tag="p_g")
            p_c = state.tile([128, NCH, B, 3, 64], f16, tag="p_c")
            x0T_l0 = [state.tile([128, NCH, 128], f16, tag=f"x0T_l0_{b}", name=f"x0T_l0_{b}") for b in range(B)]
            x0T_l1 = [state.tile([128, NCH, 128], f16, tag=f"x0T_l1_{b}", name=f"x0T_l1_{b}") for b in range(B)]
            x0cT_l0 = [state.tile([128, NCH, 128], f16, tag=f"x0cT_l0_{b}", name=f"x0cT_l0_{b}") for b in range(B)]
            x0cT_l1 = [state.tile([128, NCH, 128], f16, tag=f"x0cT_l1_{b}", name=f"x0cT_l1_{b}") for b in range(B)]
            rhT_pair = [state.tile([128, NCH, 128], f16, tag=f"rhT_{p}", name=f"rhT_{p}") for p in range(2)]
            yT = [state.tile([1, N], f16, tag=f"yT_{b}", name=f"yT_{b}") for b in range(B)]
            ysb = [state.tile([1, N], f32, tag=f"ysb_{b}", name=f"ysb_{b}") for b in range(B)]

            for t_ in ([h_all] + x0T_l0 + x0T_l1 + x0cT_l0 + x0cT_l1 + yT):
                nc.vector.memset(t_, 0.0)
            for t_ in x0T_l0 + x0cT_l0:
                nc.vector.memset(t_[32:33, :, :], 1.0)

            def gconv(x0T, psb, wS, bS, wi, O, act, outt, skip_bias=False):
                """pk projections + fused diffusion sweep + combine.
                x0T: per-b feature-major lhsT tiles; psb: SBUF (128,NCH,B,3,O)
                fp16 landing pad; wS/bS: weight/bias const tiles; O: 128|64;
                act: Sigmoid|Tanh; outt: (128,NCH,B,O) fp16 output."""
                for half in range(2):
                    for b in range(B):
                        for mc in range(half * 4, half * 4 + 4):
                            ps = pkps.tile([128, 3 * O], f32, tag="pk")
                            nc.tensor.matmul(ps, x0T[b][:, mc, :],
                                             wS[:, wi, :], start=True, stop=True)
                            if (b * 4 + mc) % 2 == 0:
                                nc.scalar.copy(psb[:, mc, b, :, :], ps)
                            else:
                                nc.vector.tensor_copy(psb[:, mc, b, :, :], ps)
                for mc in range(NCH):
                    sw = swps.tile([128, B, O], f32, tag="sw")
                    for kk in range(2 * NCH):
                        lhs = (sST if kk < NCH else sS2T)[:, kk % NCH, mc, :]
                        rhs = psb[:, kk % NCH, :, 1 if kk < NCH else 2, :]
                        nc.tensor.matmul(sw, lhs, rhs, start=(kk == 0),
                                         stop=(kk == 2 * NCH - 1))
                    t1 = tmp.tile([128, B, O], f32, tag="t1")
                    nc.vector.tensor_add(t1, sw, psb[:, mc, :, 0, :])
                    if skip_bias:
                        nc.scalar.activation(outt[:, mc, :, :], t1, act)
                    else:
                        t2 = tmp.tile([128, B, O], f32, tag="t2")
                        nc.vector.tensor_add(t2, t1, bc_b(bS[:, wi, :]))
                        nc.scalar.activation(outt[:, mc, :, :], t2, act)

            def cell(l, step, tail):
                enc = step < SEQ
                wi = (0 if enc else 2) + l
                dx = (2 if enc else 1) if l == 0 else 64
                x0T = x0T_l0 if l == 0 else x0T_l1
                x0cT = x0cT_l0 if l == 0 else x0cT_l1
                # ---- gate ----
                gconv(x0T, p_g, sWG, sBG, wi, 128, AF.Sigmoid, ru, skip_bias=(l == 0))
                # ---- candidate input r*h (+ transposes), per chunk-half so
                # the first half is ready while the gate sweep still runs ----
                if l == 0:
                    for half in range(2):
                        hs = slice(half * 4, half * 4 + 4)
                        for pr in range(2):
                            for b in (2 * pr, 2 * pr + 1):
                                nc.vector.tensor_mul(
                                    rh[:, b // 2, hs, b % 2, :],
                                    ru[:, hs, b, 0:U], h_all[:, b, hs, 0, :])
                            nc.sync.dma_start_transpose(
                                rhT_pair[pr][:, hs, :], rh[:, pr, hs, :, :])
                            for b in (2 * pr, 2 * pr + 1):
                                nc.vector.tensor_copy(
                                    x0cT_l0[b][64:128, hs, :],
                                    rhT_pair[pr][(b % 2) * 64:(b % 2) * 64 + 64, hs, :])
                    for b in range(B):
                        nc.vector.tensor_copy(x0cT_l0[b][0:dx, :, :],
                                              x0T_l0[b][0:dx, :, :])
                else:
                    for half in range(2):
                        hs = slice(half * 4, half * 4 + 4)
                        for b in range(B):
                            nc.vector.tensor_copy(rhx[:, b, hs, 0, :],
                                                  h_all[:, b, hs, 0, :])
                            nc.vector.tensor_mul(rhx[:, b, hs, 1, :],
                                                 ru[:, hs, b, 0:U],
                                                 h_all[:, b, hs, 1, :])
                            nc.sync.dma_start_transpose(
                                x0cT_l1[b][:, hs, :], rhx[:, b, hs, :, :])
                # ---- candidate ----
                gconv(x0cT, p_c, sWC, sBC, wi, 64, AF.Tanh, cc, skip_bias=(l == 0))
                # ---- h' = c + u*(h-c), per (b, chunk-half): the first half
                # finalizes+transposes while the cand sweep's tail runs ----
                for half in range(2):
                    hs = slice(half * 4, half * 4 + 4)
                    for b in range(B):
                        hsl = h_all[:, b, hs, l, :]
                        usl = ru[:, hs, b, U:128]
                        csl = cc[:, hs, b, :]
                        d = tmp.tile([128, 4, U], f32, tag="fd")
                        nc.vector.tensor_sub(d, hsl, csl)
                        e = tmp.tile([128, 4, U], f32, tag="fe")
                        nc.vector.tensor_mul(e, d, usl)
                        nc.vector.tensor_add(hsl, e, csl)
                        tail(b, half, hs)

            for step in range(n_steps):
                enc = step < SEQ
                if enc:
                    for b in range(B):
                        nc.sync.dma_start(out=x0T_l0[b][0:2, :, :], in_=dxT[step, b].rearrange('f (c n) -> f c n', c=NCH))
                else:
                    for b in range(B):
                        nc.vector.tensor_copy(x0T_l0[b][0:1, :, :], yT[b].rearrange('o (c n) -> o c n', c=NCH))

                def tail0(b, half, hs):
                    # x0T_l1 <- [h'_l0(s)^T ; h_l1(s-1)^T]: l1 gate input
                    nc.sync.dma_start_transpose(x0T_l1[b][:, hs, :],
                                                h_all[:, b, hs, :, :])

                cell(0, step, tail0)

                td = step - SEQ

                def tail1(b, half, hs):
                    if not enc:
                        # refresh to get h'_l1(s)^T for the projection
                        nc.sync.dma_start_transpose(x0T_l1[b][:, hs, :],
                                                    h_all[:, b, hs, :, :])
                        pp = pjps.tile([1, 512], f32, tag="pj")
                        nc.tensor.matmul(pp, sPW, x0T_l1[b][:, hs, :],
                                         start=True, stop=True)
                        nc.vector.tensor_scalar_add(
                            ysb[b][:, half * 512:(half + 1) * 512], pp, sPB)
                        nc.vector.tensor_copy(yT[b][:, half * 512:(half + 1) * 512],
                                              ysb[b][:, half * 512:(half + 1) * 512])
                        if half == 1:
                            nc.sync.dma_start(out=dY[td, b:b + 1, :], in_=ysb[b])
                    # h'_l0(s)^T for next step's l0 gate
                    nc.vector.tensor_copy(x0T_l0[b][64:128, hs, :],
                                          x0T_l1[b][0:64, hs, :])

                cell(1, step, tail1)

            if dbg:
                nc.sync.dma_start(out=dDH[:], in_=h_all)
                nc.sync.dma_start(out=dDRU[:], in_=ru)
                nc.sync.dma_start(out=dDPG[:], in_=p_g)
                nc.sync.dma_start(out=dDX1[:], in_=x0T_l1[0])

    _fix_multi_waits(nc)
    return nc


def _fix_multi_waits(nc):
    """This container's walrus accepts only ONE sync-wait per CTRL/DMA
    instruction encoding; hoist extra waits onto same-engine NoOps."""
    from concourse import mybir
    for bb in nc.main_func.blocks:
        new = []
        dirty = False
        for inst in bb.instructions:
            si = inst.sync_info
            if si is not None and len(si.on_wait) > 1:
                waits = list(si.on_wait)
                for j, w in enumerate(waits[:-1]):
                    nop = mybir.InstNoOp(name=f"{inst.name}-wsplit{j}", ins=[], outs=[])
                    nop.engine = inst.engine
                    nop.sync_info = mybir.SyncInfo(on_wait=[w], on_update=[])
                    nc.register_instruction(nop, overwrite=True)
                    new.append(nop)
                si.on_wait = waits[-1:]
                dirty = True
            new.append(inst)
        if dirty:
            bb.instructions = new


def kernel(inputs, support, params, trace=False):
    from concourse.bass_utils import run_bass_kernel_spmd

    key = 'prog'
    if key not in _CACHE:
        _CACHE[key] = _build_program()
    nc = _CACHE[key]

    in_maps = _prep_host(inputs, support, params)
    res = run_bass_kernel_spmd(nc, in_maps, list(range(NCORES)), trace=trace)
    if trace:
        kernel.last_result = res
    out = np.concatenate([np.asarray(res.results[c]["Y"]) for c in range(NCORES)],
                         axis=1)
    return out.reshape(HOR, BTOT, N).astype(np.float32)


# revision 25
# speedup vs baseline: 1.1362x; 1.0397x over previous
"""DCRNN (diffusion-conv GRU encoder-decoder) Trainium2 kernel.

Strategy (8 NeuronCores, data-parallel over batch B=32 -> 4/core):
  * Everything SBUF-resident: support (as S^T and (S^2)^T, fp16), all
    weights, hidden states. HBM traffic is only the tiny per-step input
    rows and output rows.
  * Commuted gconv: for Chebyshev terms x0,x1,x2 and per-hop weights
    W0,W1,W2 (torch row-interleaved f*3+k),
        gconv(v) = v(W0-W2) + S (v W1) + S^2 (v 2W2) + b
    so the feature projection (tiny K<=128 matmul) happens FIRST and the
    two diffusion applications become ONE fused 16-K-chunk PE sweep over
    [S^T | S2^T] with no mid-sweep dependency. S^2 is precomputed on host.
  * Layouts: node-major (node-chunk on partitions) for diffusion sweeps
    and all elementwise ops; feature-major x0^T tiles (built with xbar
    DMA-transposes, off the PE) serve as stationary lhsT for the
    projections. PE matmuls are fp16 (1 col/cycle) with fp32 PSUM.
"""

import numpy as np

SEQ, HOR, N, BTOT, U, K = 12, 12, 1024, 32, 64, 2
NCORES = 8
B = BTOT // NCORES          # 4 batch / core
NCH = N // 128              # 8 node chunks

_CACHE = {}


def _split_w(W, F):
    W = np.asarray(W, np.float32)
    return W[0::3], W[1::3], W[2::3]


def _pack_gate(Wg, F, dx, out_o, bias=None):
    """(3F, O) torch-interleaved -> (128, 3*O) fp16 [Wq | W1 | 2W2] with
    x-features at rows 0:dx, h-features at rows 64:128, and (layer-0
    cells only) the bias at row 32, matched by a constant-1.0 feature
    row in the lhsT tiles."""
    W0, W1, W2 = _split_w(Wg, F)
    cat = np.concatenate([W0 - W2, W1, 2.0 * W2], axis=1)  # (F, 3*O)
    out = np.zeros((128, 3 * out_o), np.float32)
    out[0:dx] = cat[0:dx]
    out[64:128] = cat[dx:dx + 64]
    if bias is not None:
        out[32, 0:out_o] = np.asarray(bias, np.float32)
    return out.astype(np.float16)


def _prep_host(inputs, support, params):
    S = np.asarray(support, np.float32)
    S2 = S @ S
    ST = np.ascontiguousarray(S.T).reshape(NCH, 128, NCH, 128)
    ST = np.ascontiguousarray(ST.transpose(1, 0, 2, 3)).reshape(128, NCH * NCH * 128)
    S2T = np.ascontiguousarray(S2.T).reshape(NCH, 128, NCH, 128)
    S2T = np.ascontiguousarray(S2T.transpose(1, 0, 2, 3)).reshape(128, NCH * NCH * 128)

    cells = [params['enc'][0], params['enc'][1], params['dec'][0], params['dec'][1]]
    dxs = [2, 64, 1, 64]
    WG = np.stack([_pack_gate(c[0], dx + 64, dx, 128,
                              bias=(c[1] if dx <= 2 else None))
                   for c, dx in zip(cells, dxs)], 1)
    WC = np.stack([_pack_gate(c[2], dx + 64, dx, 64,
                              bias=(c[3] if dx <= 2 else None))
                   for c, dx in zip(cells, dxs)], 1)
    BG = np.stack([np.broadcast_to(np.asarray(c[1], np.float32), (128, 128))
                   for c in cells], 1)
    BC = np.stack([np.broadcast_to(np.asarray(c[3], np.float32), (128, 64))
                   for c in cells], 1)
    PW = np.zeros((128, 1), np.float32)
    PW[64:128, 0] = np.asarray(params['proj_W'], np.float32)[:, 0]
    PB = np.asarray(params['proj_b'], np.float32).reshape(1, 1)

    # encoder inputs, feature-major per (t, b): (12, B, 2, N) per core
    x = np.asarray(inputs, np.float32).reshape(SEQ, BTOT, N, 2)
    xT = np.ascontiguousarray(x.transpose(0, 1, 3, 2)).astype(np.float16)

    shared = {
        'ST': ST.astype(np.float16), 'S2T': S2T.astype(np.float16),
        'WG': np.ascontiguousarray(WG).reshape(128, -1),
        'WC': np.ascontiguousarray(WC).reshape(128, -1),
        'BG': np.ascontiguousarray(BG).reshape(128, -1),
        'BC': np.ascontiguousarray(BC).reshape(128, -1),
        'PW': PW.astype(np.float16), 'PB': PB,
    }
    in_maps = []
    for c in range(NCORES):
        m = dict(shared)
        m['xT'] = np.ascontiguousarray(xT[:, c * B:(c + 1) * B])
        in_maps.append(m)
    return in_maps


def _build_program_dbg(n_steps):
    return _build_program(n_steps, dbg=True)


def _build_program(n_steps=SEQ + HOR, dbg=False):
    import concourse.bass as bass
    import concourse.tile as tile
    from concourse import mybir

    f32, f16 = mybir.dt.float32, mybir.dt.float16
    AF = mybir.ActivationFunctionType

    nc = bass.Bass()
    dST = nc.declare_dram_parameter("ST", [128, NCH * NCH * 128], f16, isOutput=False)
    dS2T = nc.declare_dram_parameter("S2T", [128, NCH * NCH * 128], f16, isOutput=False)
    dWG = nc.declare_dram_parameter("WG", [128, 4 * 384], f16, isOutput=False)
    dWC = nc.declare_dram_parameter("WC", [128, 4 * 192], f16, isOutput=False)
    dBG = nc.declare_dram_parameter("BG", [128, 4 * 128], f32, isOutput=False)
    dBC = nc.declare_dram_parameter("BC", [128, 4 * 64], f32, isOutput=False)
    dPW = nc.declare_dram_parameter("PW", [128, 1], f16, isOutput=False)
    dPB = nc.declare_dram_parameter("PB", [1, 1], f32, isOutput=False)
    dxT = nc.declare_dram_parameter("xT", [SEQ, B, 2, N], f16, isOutput=False)
    dY = nc.declare_dram_parameter("Y", [HOR, B, N], f32, isOutput=True)
    if dbg:
        dDH = nc.declare_dram_parameter("DH", [128, B * NCH * 2 * U], f16, isOutput=True)
        dDRU = nc.declare_dram_parameter("DRU", [128, NCH * B * 128], f16, isOutput=True)
        dDPG = nc.declare_dram_parameter("DPG", [128, NCH * B * 3 * 128], f16, isOutput=True)
        dDX1 = nc.declare_dram_parameter("DX1", [128, N], f16, isOutput=True)

    def bc_b(ap, nb=B):
        """insert a stride-0 batch dim after the partition dim"""
        return bass.AP(tensor=ap.tensor, offset=ap.offset,
                       ap=[list(ap.ap[0]), [0, nb]] + [list(a) for a in ap.ap[1:]])

    def bc_b2(ap, nb=B):
        """stride-0 (chunk-pair, batch) dims after the partition dim"""
        return bass.AP(tensor=ap.tensor, offset=ap.offset,
                       ap=[list(ap.ap[0]), [0, 2], [0, nb]] + [list(a) for a in ap.ap[1:]])

    with tile.TileContext(nc) as tc:
        import contextlib
        with contextlib.ExitStack() as ctx:
            const = ctx.enter_context(tc.tile_pool(name="const", bufs=1))
            state = ctx.enter_context(tc.tile_pool(name="state", bufs=1))
            pkps = ctx.enter_context(tc.tile_pool(name="pkps", bufs=4, space="PSUM"))
            swps = ctx.enter_context(tc.tile_pool(name="swps", bufs=2, space="PSUM"))
            pjps = ctx.enter_context(tc.tile_pool(name="pjps", bufs=2, space="PSUM"))
            tmp = ctx.enter_context(tc.tile_pool(name="tmp", bufs=3))

            sST = const.tile([128, NCH, NCH, 128], f16)
            nc.sync.dma_start(out=sST, in_=dST[:])
            sS2T = const.tile([128, NCH, NCH, 128], f16)
            nc.sync.dma_start(out=sS2T, in_=dS2T[:])
            sWG = const.tile([128, 4, 384], f16)
            nc.sync.dma_start(out=sWG, in_=dWG[:])
            sWC = const.tile([128, 4, 192], f16)
            nc.sync.dma_start(out=sWC, in_=dWC[:])
            sBG = const.tile([128, 4, 128], f32)
            nc.sync.dma_start(out=sBG, in_=dBG[:])
            sBC = const.tile([128, 4, 64], f32)
            nc.sync.dma_start(out=sBC, in_=dBC[:])
            sPW = const.tile([128, 1], f16)
            nc.sync.dma_start(out=sPW, in_=dPW[:])
            sPB = const.tile([1, 1], f32)
            nc.sync.dma_start(out=sPB, in_=dPB[:])

            h_all = state.tile([128, B, NCH, 2, U], f16, tag="h_all")
            ru = state.tile([128, NCH, B, 128], f16, tag="ru")
            cc = state.tile([128, NCH, B, U], f16, tag="cc")
            rh = state.tile([128, 2, NCH, 2, U], f16, tag="rh")
            rhx = state.tile([128, B, NCH, 2, U], f16, tag="rhx")
            p_g = state.tile([128, NCH, B, 3, 128], f16, tag="p_g")
            p_c = state.tile([128, NCH, B, 3, 64], f16, tag="p_c")
            x0T_l0 = [state.tile([128, NCH, 128], f16, tag=f"x0T_l0_{b}", name=f"x0T_l0_{b}") for b in range(B)]
            x0T_l1 = [state.tile([128, NCH, 128], f16, tag=f"x0T_l1_{b}", name=f"x0T_l1_{b}") for b in range(B)]
            x0cT_l0 = [state.tile([128, NCH, 128], f16, tag=f"x0cT_l0_{b}", name=f"x0cT_l0_{b}") for b in range(B)]
            x0cT_l1 = [state.tile([128, NCH, 128], f16, tag=f"x0cT_l1_{b}", name=f"x0cT_l1_{b}") for b in range(B)]
            rhT_pair = [state.tile([128, NCH, 128], f16, tag=f"rhT_{p}", name=f"rhT_{p}") for p in range(2)]
            yT = [state.tile([1, N], f16, tag=f"yT_{b}", name=f"yT_{b}") for b in range(B)]
            ysb = [state.tile([1, N], f32, tag=f"ysb_{b}", name=f"ysb_{b}") for b in range(B)]

            for t_ in ([h_all] + x0T_l0 + x0T_l1 + x0cT_l0 + x0cT_l1 + yT):
                nc.vector.memset(t_, 0.0)
            for t_ in x0T_l0 + x0cT_l0:
                nc.vector.memset(t_[32:33, :, :], 1.0)

            def gconv(x0T, psb, wS, bS, wi, O, act, outt, skip_bias=False):
                """pk projections + fused diffusion sweep + combine.
                x0T: per-b feature-major lhsT tiles; psb: SBUF (128,NCH,B,3,O)
                fp16 landing pad; wS/bS: weight/bias const tiles; O: 128|64;
                act: Sigmoid|Tanh; outt: (128,NCH,B,O) fp16 output."""
                for half in range(2):
                    for b in range(B):
                        for mc in range(half * 4, half * 4 + 4):
                            ps = pkps.tile([128, 3 * O], f32, tag="pk")
                            nc.tensor.matmul(ps, x0T[b][:, mc, :],
                                             wS[:, wi, :], start=True, stop=True)
                            if (b * 4 + mc) % 2 == 0:
                                nc.scalar.copy(psb[:, mc, b, :, :], ps)
                            else:
                                nc.vector.tensor_copy(psb[:, mc, b, :, :], ps)
                for mc in range(NCH):
                    sw = swps.tile([128, B, O], f32, tag="sw")
                    for kk in range(2 * NCH):
                        lhs = (sST if kk < NCH else sS2T)[:, kk % NCH, mc, :]
                        rhs = psb[:, kk % NCH, :, 1 if kk < NCH else 2, :]
                        nc.tensor.matmul(sw, lhs, rhs, start=(kk == 0),
                                         stop=(kk == 2 * NCH - 1))
                    t1 = tmp.tile([128, B, O], f32, tag="t1")
                    nc.vector.tensor_add(t1, sw, psb[:, mc, :, 0, :])
                    if skip_bias:
                        nc.scalar.activation(outt[:, mc, :, :], t1, act)
                    else:
                        t2 = tmp.tile([128, B, O], f32, tag="t2")
                        nc.vector.tensor_add(t2, t1, bc_b(bS[:, wi, :]))
                        nc.scalar.activation(outt[:, mc, :, :], t2, act)

            def cell(l, step, tail):
                enc = step < SEQ
                wi = (0 if enc else 2) + l
                dx = (2 if enc else 1) if l == 0 else 64
                x0T = x0T_l0 if l == 0 else x0T_l1
                x0cT = x0cT_l0 if l == 0 else x0cT_l1
                # ---- gate ----
                gconv(x0T, p_g, sWG, sBG, wi, 128, AF.Sigmoid, ru, skip_bias=(l == 0))
                # ---- candidate input r*h (+ transposes), per chunk-half so
                # the first half is ready while the gate sweep still runs ----
                if l == 0:
                    for half in range(2):
                        hs = slice(half * 4, half * 4 + 4)
                        for pr in range(2):
                            for b in (2 * pr, 2 * pr + 1):
                                nc.vector.tensor_mul(
                                    rh[:, b // 2, hs, b % 2, :],
                                    ru[:, hs, b, 0:U], h_all[:, b, hs, 0, :])
                            nc.sync.dma_start_transpose(
                                rhT_pair[pr][:, hs, :], rh[:, pr, hs, :, :])
                            for b in (2 * pr, 2 * pr + 1):
                                nc.scalar.copy(
                                    x0cT_l0[b][64:128, hs, :],
                                    rhT_pair[pr][(b % 2) * 64:(b % 2) * 64 + 64, hs, :])
                    for b in range(B):
                        nc.vector.tensor_copy(x0cT_l0[b][0:dx, :, :],
                                              x0T_l0[b][0:dx, :, :])
                else:
                    for half in range(2):
                        hs = slice(half * 4, half * 4 + 4)
                        for b in range(B):
                            nc.scalar.copy(rhx[:, b, hs, 0, :],
                                                  h_all[:, b, hs, 0, :])
                            nc.vector.tensor_mul(rhx[:, b, hs, 1, :],
                                                 ru[:, hs, b, 0:U],
                                                 h_all[:, b, hs, 1, :])
                            nc.sync.dma_start_transpose(
                                x0cT_l1[b][:, hs, :], rhx[:, b, hs, :, :])
                # ---- candidate ----
                gconv(x0cT, p_c, sWC, sBC, wi, 64, AF.Tanh, cc, skip_bias=(l == 0))
                # ---- h' = c + u*(h-c), per (b, chunk-half): the first half
                # finalizes+transposes while the cand sweep's tail runs ----
                for half in range(2):
                    hs = slice(half * 4, half * 4 + 4)
                    for b in range(B):
                        hsl = h_all[:, b, hs, l, :]
                        usl = ru[:, hs, b, U:128]
                        csl = cc[:, hs, b, :]
                        d = tmp.tile([128, 4, U], f32, tag="fd")
                        nc.vector.tensor_sub(d, hsl, csl)
                        e = tmp.tile([128, 4, U], f32, tag="fe")
                        nc.vector.tensor_mul(e, d, usl)
                        nc.vector.tensor_add(hsl, e, csl)
                        tail(b, half, hs)

            for step in range(n_steps):
                enc = step < SEQ
                if enc:
                    for b in range(B):
                        nc.sync.dma_start(out=x0T_l0[b][0:2, :, :], in_=dxT[step, b].rearrange('f (c n) -> f c n', c=NCH))
                else:
                    for b in range(B):
                        nc.vector.tensor_copy(x0T_l0[b][0:1, :, :], yT[b].rearrange('o (c n) -> o c n', c=NCH))

                def tail0(b, half, hs):
                    # x0T_l1 <- [h'_l0(s)^T ; h_l1(s-1)^T]: l1 gate input
                    nc.sync.dma_start_transpose(x0T_l1[b][:, hs, :],
                                                h_all[:, b, hs, :, :])

                cell(0, step, tail0)

                td = step - SEQ

                def tail1(b, half, hs):
                    if not enc:
                        # refresh to get h'_l1(s)^T for the projection
                        nc.sync.dma_start_transpose(x0T_l1[b][:, hs, :],
                                                    h_all[:, b, hs, :, :])
                        pp = pjps.tile([1, 512], f32, tag="pj")
                        nc.tensor.matmul(pp, sPW, x0T_l1[b][:, hs, :],
                                         start=True, stop=True)
                        nc.vector.tensor_scalar_add(
                            ysb[b][:, half * 512:(half + 1) * 512], pp, sPB)
                        nc.vector.tensor_copy(yT[b][:, half * 512:(half + 1) * 512],
                                              ysb[b][:, half * 512:(half + 1) * 512])
                        if half == 1:
                            nc.sync.dma_start(out=dY[td, b:b + 1, :], in_=ysb[b])
                    # h'_l0(s)^T for next step's l0 gate
                    nc.scalar.copy(x0T_l0[b][64:128, hs, :],
                                   x0T_l1[b][0:64, hs, :])

                cell(1, step, tail1)

            if dbg:
                nc.sync.dma_start(out=dDH[:], in_=h_all)
                nc.sync.dma_start(out=dDRU[:], in_=ru)
                nc.sync.dma_start(out=dDPG[:], in_=p_g)
                nc.sync.dma_start(out=dDX1[:], in_=x0T_l1[0])

    _fix_multi_waits(nc)
    return nc


def _fix_multi_waits(nc):
    """This container's walrus accepts only ONE sync-wait per CTRL/DMA
    instruction encoding; hoist extra waits onto same-engine NoOps."""
    from concourse import mybir
    for bb in nc.main_func.blocks:
        new = []
        dirty = False
        for inst in bb.instructions:
            si = inst.sync_info
            if si is not None and len(si.on_wait) > 1:
                waits = list(si.on_wait)
                for j, w in enumerate(waits[:-1]):
                    nop = mybir.InstNoOp(name=f"{inst.name}-wsplit{j}", ins=[], outs=[])
                    nop.engine = inst.engine
                    nop.sync_info = mybir.SyncInfo(on_wait=[w], on_update=[])
                    nc.register_instruction(nop, overwrite=True)
                    new.append(nop)
                si.on_wait = waits[-1:]
                dirty = True
            new.append(inst)
        if dirty:
            bb.instructions = new


def kernel(inputs, support, params, trace=False):
    from concourse.bass_utils import run_bass_kernel_spmd

    key = 'prog'
    if key not in _CACHE:
        _CACHE[key] = _build_program()
    nc = _CACHE[key]

    in_maps = _prep_host(inputs, support, params)
    res = run_bass_kernel_spmd(nc, in_maps, list(range(NCORES)), trace=trace)
    if trace:
        kernel.last_result = res
    out = np.concatenate([np.asarray(res.results[c]["Y"]) for c in range(NCORES)],
                         axis=1)
    return out.reshape(HOR, BTOT, N).astype(np.float32)
